# revision 1
# baseline (speedup 1.0000x reference)
"""Trainium2 Bass kernel for the CovidModel scenario forecaster.

Math: the reference's 365-day lax.scan linearizes exactly.
With s(tau) = a0(tau) + eps*a1(tau) (the combined covariate):
    a_v(tau) = delta_v * u(tau) * s(tau-1),  u = rt^(1/T)
=>  s(tau)   = s(tau-1) * K * u(tau),        K = delta0 + eps*delta1
a pure cumulative product (hardware tensor_tensor_scan).  The three
Poisson-PMF window convolutions (a->m->e->out) are linear filters, so
    out(b,t) = sum_d C3[d] * q(b, t-3-d) + warmup boundary terms
with q = s/K and C3 the tap-composition of pi_G*pi_X*pi_M weighted by
rho/delta per vax status.  The warmup boundary only touches t<=30; its
(64-feature x 30-day) matrix is folded on the host together with s0
(both are O(B*64) gathers over the tiny warmup tensors).

Device pipeline per 128-scenario tile (scenarios on partitions):
  DMA rt -> ACT ln -> ACT exp(scale,bias) -> DVE scan (cumprod along
  time) -> PE transpose (time onto partitions) -> PE banded matmuls
  -> DVE copy (+warmup add) -> DMA out.
All PE inputs are produced by DVE so each Matmult carries a single
sync wait (the fused fp32 weight-load slot only fits one).
Sharding: batch B=16384 split 8 ways, pure data parallel, no
collectives; parameter-derived constants are replicated.
"""

import numpy as np

import concourse.bacc as bacc
import concourse.bass as bass
import concourse.mybir as mybir
import concourse.tile as tile
from concourse import hw_specs
from concourse.bass_utils import run_bass_kernel_spmd

# The act-table insertion pass picks the first table containing each
# activation function, which puts Ln and Exp in different LUT sets and
# forces a ~1.3us table reload whenever the ACT stream alternates between
# them.  Shrink every other set's advertised membership so both functions
# resolve to the combined natural_log_exp_and_others table (ids keep their
# positions, so the loaded table is still correct).
_orig_get_tables = hw_specs.get_activation_tables


def _patched_get_tables(arch):
    tabs = _orig_get_tables(arch)
    combined = "natural_log_exp_and_others"
    if combined in tabs:
        both = {mybir.ActivationFunctionType.Exp, mybir.ActivationFunctionType.Ln}
        if both <= tabs[combined]:
            for name, s in tabs.items():
                if name != combined:
                    s.difference_update(both)
    return tabs


bacc.get_activation_tables = _patched_get_tables

# Problem constants (fixed by the nn.Module definition)
J = 10
T_SERIAL = 5.8
B = 16384
FORECAST = 365
N_CORES = 8
B_SHARD = B // N_CORES          # 2048
N_TILES = B_SHARD // 128        # 16
TPAD = 384                      # 365 padded to 3 x 128
NCST = 3 * FORECAST + 128       # packed consts: mband chunks + identity
F32 = mybir.dt.float32


def _make_constants(eps, delta, rho_M, rho_X, rho_G, pi_M, pi_X, pi_G):
    """Fold the tiny replicated parameters into matmul constants."""
    eps, delta, rho_M, rho_X, rho_G, pi_M, pi_X, pi_G = [
        np.asarray(a, np.float64)
        for a in (eps, delta, rho_M, rho_X, rho_G, pi_M, pi_X, pi_G)
    ]
    K = delta[0] + eps[0] * delta[1]
    invT = 1.0 / T_SERIAL

    C3 = np.zeros(3 * (J - 1) + 1)
    for v in range(2):
        W = np.convolve(np.convolve(pi_G[v], pi_X[v]), pi_M[v])
        C3 += rho_G[v] * rho_X[v] * rho_M[v] * delta[v] * W
    C3n = C3 / K

    mband = np.zeros((TPAD, FORECAST))
    r = np.arange(FORECAST)[:, None]
    c = np.arange(FORECAST)[None, :]
    d = c - r - 3
    mask = (d >= 0) & (d <= 27)
    mband[:FORECAST][mask] = C3n[d[mask]]

    bm = np.zeros((64, 30))
    for v in range(2):
        for D in range(10):            # warmup day 20+D, tau = D - 9
            tau = D - 9
            for t in range(1, 31):
                col = t - 1
                j = t - 1 - tau
                if 0 <= j <= 9:
                    bm[40 + 10 * v + D, col] += rho_G[v] * pi_G[v, j]
                acc = 0.0
                for jj in range(10):
                    k = t - 2 - jj - tau
                    if 0 <= k <= 9 and (t - 1 - jj) >= 1:
                        acc += pi_G[v, jj] * pi_X[v, k]
                bm[20 + 10 * v + D, col] += rho_G[v] * rho_X[v] * acc
                acc = 0.0
                for jj in range(10):
                    for k in range(10):
                        l = t - 3 - jj - k - tau
                        if (0 <= l <= 9 and (t - 1 - jj) >= 1
                                and (t - 2 - jj - k) >= 1):
                            acc += pi_G[v, jj] * pi_X[v, k] * pi_M[v, l]
                bm[10 * v + D, col] += rho_G[v] * rho_X[v] * rho_M[v] * acc

    return (float(eps[0]), float(invT), float(np.log(K)),
            mband.astype(np.float32), bm.astype(np.float32))


def _build_nc(invT, lnK):
    nc = bacc.Bacc()

    # merged per-scenario input: col 0 = s0 (scan seed), cols 1..30 =
    # warmup contribution for t=1..30, cols 31..395 = rt
    rt_d = nc.dram_tensor("rt", [B_SHARD, 31 + FORECAST], F32,
                          kind="ExternalInput")
    # packed constants: [0:1095] mband (3 tau-chunks side by side),
    # [1095:1223] identity
    cst_d = nc.dram_tensor("cst", [128, NCST], F32, kind="ExternalInput")
    out_d = nc.dram_tensor("out", [B_SHARD, FORECAST], F32, kind="ExternalOutput")

    Exp = mybir.ActivationFunctionType.Exp
    Ln = mybir.ActivationFunctionType.Ln

    with tile.TileContext(nc) as tc:
        with (
            tc.tile_pool(name="consts", bufs=1) as consts,
            tc.tile_pool(name="rt", bufs=6) as rt_pool,
            tc.tile_pool(name="work", bufs=4) as work,
            tc.tile_pool(name="st", bufs=4) as st_pool,
            tc.tile_pool(name="outp", bufs=4) as out_pool,
            tc.tile_pool(name="stp", bufs=3, space=bass.MemorySpace.PSUM) as st_psum,
            tc.tile_pool(name="op", bufs=4, space=bass.MemorySpace.PSUM) as out_psum,
        ):
            # stage constants through DVE so PE consumers sync on one sem
            cst_ld = consts.tile([128, NCST], F32, tag="cst_ld")
            nc.gpsimd.dma_start(cst_ld[:], cst_d[:])
            cst = consts.tile([128, NCST], F32, tag="cst")
            nc.vector.tensor_copy(cst[:], cst_ld[:])
            mb_sb = cst[:, 0:3 * FORECAST]
            ident = cst[:, 3 * FORECAST:]
            lnk_sb = consts.tile([128, 1], F32, tag="lnk")
            nc.gpsimd.memset(lnk_sb[:], float(lnK))

            for i in range(N_TILES):
                rows = slice(i * 128, (i + 1) * 128)
                # tile layout: [0]=s0, [1:31]=wc, [31:396]=rt, [396:415]=pad
                rt_t = rt_pool.tile([128, 31 + TPAD], F32, tag="rt")
                nc.scalar.dma_start(rt_t[:, :31 + FORECAST], rt_d[rows, :])
                nc.gpsimd.memset(rt_t[:, 31 + FORECAST:], 1.0)

                # f = exp(invT * ln(rt) + lnK); pad cols give f = K (finite)
                lr_t = work.tile([128, TPAD], F32, tag="lr")
                nc.scalar.activation(lr_t[:], rt_t[:, 31:], Ln)
                f_t = work.tile([128, TPAD], F32, tag="f")
                nc.scalar.activation(
                    f_t[:], lr_t[:], Exp, bias=lnk_sb[:, 0:1], scale=float(invT))

                # s(tau) cumulative product along time, seeded with s0
                s_t = work.tile([128, TPAD], F32, tag="s")
                nc.vector.tensor_tensor_scan(
                    s_t[:], f_t[:], f_t[:], rt_t[:, 0:1],
                    op0=mybir.AluOpType.mult, op1=mybir.AluOpType.bypass)

                # transpose the 3 time chunks onto partitions
                stp = st_psum.tile([128, TPAD], F32, tag="stp")
                for chunk in range(3):
                    cs = slice(chunk * 128, (chunk + 1) * 128)
                    nc.tensor.transpose(stp[:, cs], s_t[:, cs], ident[:])
                st_sb = st_pool.tile([128, TPAD], F32, tag="st")
                nc.vector.tensor_copy(st_sb[:], stp[:])

                # banded matmuls: out(b,t) = sum_tau sT(tau,b)*mband(tau,t)
                op = out_psum.tile([128, FORECAST], F32, tag="op")
                nc.tensor.matmul(
                    op[:], st_sb[:, 0:128], mb_sb[:, 0:FORECAST],
                    start=True, stop=False)
                # chunk 1: tau 129..256 -> t in [132,286] -> cols 131..285
                nc.tensor.matmul(
                    op[:, 131:286], st_sb[:, 128:256],
                    mb_sb[:, FORECAST + 131:FORECAST + 286],
                    start=False, stop=False)
                # chunk 2: tau 257..365 -> t in [260,365] -> cols 259..364
                nc.tensor.matmul(
                    op[:, 259:365], st_sb[:, 256:384],
                    mb_sb[:, 2 * FORECAST + 259:2 * FORECAST + 365],
                    start=False, stop=True)

                # out = psum (+ warmup contribution on the first 30 days)
                o_sb = out_pool.tile([128, FORECAST], F32, tag="o")
                nc.vector.tensor_add(o_sb[:, 0:30], op[:, 0:30], rt_t[:, 1:31])
                nc.vector.tensor_copy(o_sb[:, 30:], op[:, 30:])
                nc.sync.dma_start(out_d[rows, :], o_sb[:])

    nc.compile()
    return nc


_CACHE = {}


def _prep(inputs):
    """Returns (nc, in_maps) for the given full-size inputs."""
    r_t = np.ascontiguousarray(np.asarray(inputs["r_t"], np.float32))
    wa = np.asarray(inputs["warmup_asymp"], np.float32)
    wm = np.asarray(inputs["warmup_mild"], np.float32)
    we = np.asarray(inputs["warmup_extreme"], np.float32)

    eps, invT, lnK, mband, bm = _make_constants(
        inputs["eps"], inputs["delta"], inputs["rho_M"], inputs["rho_X"],
        inputs["rho_G"], inputs["pi_M"], inputs["pi_X"], inputs["pi_G"])

    key = (round(lnK, 12), round(invT, 12))
    if key not in _CACHE:
        _CACHE[key] = _build_nc(invT, lnK)
    nc = _CACHE[key]

    # warmup features: last 10 days of each compartment, (B, 64)
    wfeat = np.zeros((B, 64), np.float32)
    for ci, arr in enumerate((wa, wm, we)):
        for v in range(2):
            wfeat[:, 20 * ci + 10 * v: 20 * ci + 10 * v + 10] = arr[v, :, 20:30]
    merged = np.empty((B, 31 + FORECAST), np.float32)
    merged[:, 0] = wfeat[:, 9] + np.float32(eps) * wfeat[:, 19]   # s0 seed
    merged[:, 1:31] = wfeat @ bm         # (B, 30) warmup boundary terms
    merged[:, 31:] = r_t

    cstpack = np.zeros((128, NCST), np.float32)
    cstpack[:, :3 * FORECAST] = (
        mband.reshape(3, 128, FORECAST).transpose(1, 0, 2).reshape(128, -1))
    cstpack[:, 3 * FORECAST:] = np.eye(128, dtype=np.float32)

    in_maps = []
    for c in range(N_CORES):
        rows = slice(c * B_SHARD, (c + 1) * B_SHARD)
        in_maps.append({
            "rt": merged[rows],
            "cst": cstpack,
        })
    return nc, in_maps


def kernel(**inputs):
    nc, in_maps = _prep(inputs)
    res = run_bass_kernel_spmd(nc, in_maps, list(range(N_CORES)))
    return np.concatenate([res.results[c]["out"] for c in range(N_CORES)], axis=0)



# revision 3
# speedup vs baseline: 1.7400x; 1.7400x over previous
"""Trainium2 Bass kernel for the CovidModel scenario forecaster (v2).

Math: the reference's 365-day lax.scan linearizes exactly.  With
s(tau) = a0(tau) + eps*a1(tau), s(tau) = s0 * K^tau * exp(invT * P(tau))
where P(tau) = sum_{u<=tau} ln rt_u and K = delta0 + eps*delta1.  The
three Poisson-window convolutions compose into one 28-tap linear filter
C3 on s, plus a warmup boundary term (host-folded 64x30 matrix, as in
v1).

v2 layout: TIME ON PARTITIONS (365 days -> 3 chunks of 128).  The
cumulative sum P becomes 6 blocked PE matmuls (lower-triangular L for
the diagonal blocks, all-ONES for the chunk-carry blocks) over
lnrt[tau, b]; ACT computes s' = exp(invT*P + bias_tau) from PSUM; DVE
multiplies in the per-scenario seed (s0, partition-broadcast once); the
band filter is 5 more Toeplitz matmuls + 1 warmup matmul; the out pass
rescales rows by e^{c*t} (ACT/DVE split) and the result leaves as one
fp16 stream per chunk.  A drift renormalization s'(tau) =
s(tau)/(s0*e^{c*tau}) (c = mean daily log-growth, folded into the exp
bias, the Toeplitz taps and the out row-scale) keeps every 16-bit
tensor in comfortable fp16 range; everything on the wires is fp16, all
matmul moving operands are fp16 (1 PE cycle/row vs 4 for fp32).

Host prep: shard/transpose rt, elementwise ln(rt) re-encoding, the tiny
warmup folds (O(B*64)), and the final unscale-transpose.  All scan /
convolution / exp compute runs on device.

Sharding: batch 16384 split 8 ways, pure data parallel, no collectives.
"""

import numpy as np

import concourse.bacc as bacc
import concourse.bass as bass
import concourse.mybir as mybir
import concourse.tile as tile
from concourse.bass_utils import run_bass_kernel_spmd

# Problem constants (fixed by the nn.Module definition)
J = 10
T_SERIAL = 5.8
B = 16384
FORECAST = 365
N_CORES = 8
BS = B // N_CORES               # 2048 scenarios per core
NCH = 3                         # 365 days -> 3 chunks of 128 (19 pad rows)
TPAD = NCH * 128
INVT = 1.0 / T_SERIAL
SCALE_OUT = 2.0 ** -8           # device output is out * SCALE_OUT

F16 = mybir.dt.float16
F32 = mybir.dt.float32
Exp = mybir.ActivationFunctionType.Exp
Copy = mybir.ActivationFunctionType.Copy
MULT = mybir.AluOpType.mult

# cst16 column layout: [L | ONES | Adiag | Abound | Aw]
CL, CO, CD, CB, CW = (slice(128 * k, 128 * (k + 1)) for k in range(5))


def _make_host_constants(eps, delta, rho_M, rho_X, rho_G, pi_M, pi_X, pi_G,
                         lnrt_mean):
    """Fold the tiny replicated parameters into device matrices."""
    eps, delta, rho_M, rho_X, rho_G, pi_M, pi_X, pi_G = [
        np.asarray(a, np.float64)
        for a in (eps, delta, rho_M, rho_X, rho_G, pi_M, pi_X, pi_G)
    ]
    K = delta[0] + eps[0] * delta[1]
    lnK = np.log(K)
    c_drift = lnK + INVT * lnrt_mean

    C3 = np.zeros(3 * (J - 1) + 1)
    for v in range(2):
        W = np.convolve(np.convolve(pi_G[v], pi_X[v]), pi_M[v])
        C3 += rho_G[v] * rho_X[v] * rho_M[v] * delta[v] * W
    C3n = C3 / K

    p = np.arange(128)[:, None]
    i = np.arange(128)[None, :]

    def band_block(off):
        A = np.zeros((128, 128))
        d = off + i - p - 3
        m = (d >= 0) & (d <= 27)
        A[m] = C3n[d[m].astype(int)] * np.exp(-c_drift * (d[m] + 3))
        return A

    # warmup boundary matrix bm [64, 30] (same folding as v1)
    bm = np.zeros((64, 30))
    for v in range(2):
        for D in range(10):
            tau = D - 9
            for t in range(1, 31):
                col = t - 1
                j = t - 1 - tau
                if 0 <= j <= 9:
                    bm[40 + 10 * v + D, col] += rho_G[v] * pi_G[v, j]
                acc = 0.0
                for jj in range(10):
                    k = t - 2 - jj - tau
                    if 0 <= k <= 9 and (t - 1 - jj) >= 1:
                        acc += pi_G[v, jj] * pi_X[v, k]
                bm[20 + 10 * v + D, col] += rho_G[v] * rho_X[v] * acc
                acc = 0.0
                for jj in range(10):
                    for k in range(10):
                        ll = t - 3 - jj - k - tau
                        if (0 <= ll <= 9 and (t - 1 - jj) >= 1
                                and (t - 2 - jj - k) >= 1):
                            acc += pi_G[v, jj] * pi_X[v, k] * pi_M[v, ll]
                bm[10 * v + D, col] += rho_G[v] * rho_X[v] * rho_M[v] * acc

    cst16 = np.zeros((128, 5 * 128), np.float16)
    cst16[:, CL] = np.triu(np.ones((128, 128)))      # W[p,i]=1 iff p<=i
    cst16[:, CO] = 1.0
    cst16[:, CD] = band_block(0).astype(np.float16)
    cst16[:, CB] = band_block(128).astype(np.float16)
    Aw = np.zeros((64, 128))
    Aw[:, :30] = bm * np.exp(-c_drift * (np.arange(30) + 1))[None, :]
    cst16[:64, CW] = Aw.astype(np.float16)

    tau_idx = np.arange(1, TPAD + 1, dtype=np.float64)
    cst32 = np.zeros((128, 6), np.float32)
    bias = -INVT * lnrt_mean * tau_idx               # exp bias per tau
    rs = np.exp(c_drift * tau_idx)                   # out row-scale per t row
    for c in range(NCH):
        cst32[:, c] = bias[c * 128:(c + 1) * 128]
        cst32[:, 3 + c] = rs[c * 128:(c + 1) * 128]
    return cst16, cst32


def _build_nc():
    nc = bacc.Bacc()

    lnrt_d = nc.dram_tensor("lnrt", [128, NCH * BS], F16, kind="ExternalInput")
    warm_d = nc.dram_tensor("warm", [64, BS], F16, kind="ExternalInput")
    s0_d = nc.dram_tensor("s0row", [1, BS], F16, kind="ExternalInput")
    c16_d = nc.dram_tensor("cst16", [128, 5 * 128], F16, kind="ExternalInput")
    c32_d = nc.dram_tensor("cst32", [128, 6], F32, kind="ExternalInput")
    out_d = nc.dram_tensor("out", [128, NCH * BS], F16, kind="ExternalOutput")

    H = BS // 2                  # 1024-col half for the out passes

    with tile.TileContext(nc) as tc:
        with (
            tc.tile_pool(name="sb", bufs=1) as sb,
            tc.tile_pool(name="ps", bufs=2, space=bass.MemorySpace.PSUM) as ps,
        ):
            # --- input DMAs (SP queue; no waits so they stream back-to-back)
            c16 = sb.tile([128, 5 * 128], F16, tag="c16")
            nc.sync.dma_start(c16[:], c16_d[:])
            c32 = sb.tile([128, 6], F32, tag="c32")
            nc.sync.dma_start(c32[:], c32_d[:])
            s0r = sb.tile([1, BS], F16, tag="s0r")
            nc.sync.dma_start(s0r[:], s0_d[:])
            lnf = sb.tile([128, NCH * BS], F16, tag="lnf")
            for h in range(2 * NCH):     # 1024-col pieces for smooth feed
                sl = slice(h * H, (h + 1) * H)
                nc.sync.dma_start(lnf[:, sl], lnrt_d[:, sl])
            warm = sb.tile([64, BS], F16, tag="warm")
            nc.sync.dma_start(warm[:], warm_d[:])

            # --- S0 broadcast across partitions (Pool; off critical path)
            S0t = sb.tile([128, BS], F16, tag="S0t")
            nc.gpsimd.partition_broadcast(S0t[:], s0r[:])

            Lw = c16[:, CL]
            Ow = c16[:, CO]
            Dw = c16[:, CD]
            Bw = c16[:, CB]
            Ww = c16[:64, CW]

            def mm(out_ap, w_ap, x_ap, start, stop):
                nc.tensor.matmul(out_ap, w_ap, x_ap, start=start, stop=stop)

            # --- prefix-sum matmuls, P_c = L.lnf_c + sum_{c'<c} ONES.lnf_c'
            P = []
            for c in range(NCH):
                Pc = ps.tile([128, BS], F32, tag="ps", name=f"P{c}")
                P.append(Pc)
                for j in range(4):
                    sl = slice(j * 512, (j + 1) * 512)
                    first = True
                    for cp in range(c):
                        mm(Pc[:, sl], Ow, lnf[:, cp * BS:][:, sl],
                           start=first, stop=False)
                        first = False
                    mm(Pc[:, sl], Lw, lnf[:, c * BS:][:, sl],
                       start=first, stop=True)

            # --- exp + seed multiply (chunk c ready as soon as P_c is)
            s1 = []
            s2 = []
            for c in range(NCH):
                s1c = sb.tile([128, BS], F16, tag=f"s1_{c}")
                nc.scalar.activation(s1c[:], P[c][:], Exp,
                                     bias=c32[:, c:c + 1], scale=float(INVT))
                s1.append(s1c)
                s2c = sb.tile([128, BS], F16, tag=f"s2_{c}")
                nc.vector.tensor_tensor(s2c[:], s1c[:], S0t[:], op=MULT)
                s2.append(s2c)

            # --- band matmuls + out passes per chunk
            out_sb = []
            for c in range(NCH):
                bd = ps.tile([128, BS], F32, tag="ps", name=f"bd{c}")
                for j in range(4):
                    sl = slice(j * 512, (j + 1) * 512)
                    mm(bd[:, sl], Dw, s2[c][:, sl], start=True, stop=False)
                    if c == 0:
                        mm(bd[:, sl], Ww, warm[:, sl], start=False, stop=True)
                    else:
                        mm(bd[:, sl], Bw, s2[c - 1][:, sl],
                           start=False, stop=True)

                o = sb.tile([128, BS], F16, tag=f"o{c}")
                rs = c32[:, 3 + c:4 + c]
                # split the rescale copy across ACT and DVE
                nc.scalar.activation(o[:, 0:H], bd[:, 0:H], Copy,
                                     bias=0.0, scale=rs)
                nc.vector.tensor_scalar_mul(o[:, H:BS], bd[:, H:BS], rs)
                out_sb.append(o)
                nc.sync.dma_start(out_d[:, c * BS:(c + 1) * BS], o[:])

    nc.compile()
    return nc


_CACHE = {}


def _prep(inputs):
    r_t = np.asarray(inputs["r_t"], np.float32)
    wa = np.asarray(inputs["warmup_asymp"], np.float32)
    wm = np.asarray(inputs["warmup_mild"], np.float32)
    we = np.asarray(inputs["warmup_extreme"], np.float32)
    eps = float(np.asarray(inputs["eps"], np.float64)[0])

    lnrt = np.log(r_t)                       # [B, 365] fp32
    lnrt_mean = float(lnrt.astype(np.float64).mean())

    cst16, cst32 = _make_host_constants(
        inputs["eps"], inputs["delta"], inputs["rho_M"], inputs["rho_X"],
        inputs["rho_G"], inputs["pi_M"], inputs["pi_X"], inputs["pi_G"],
        lnrt_mean)

    if "nc" not in _CACHE:
        _CACHE["nc"] = _build_nc()
    nc = _CACHE["nc"]

    # warmup features: last 10 days of each compartment, (B, 64)
    wfeat = np.zeros((B, 64), np.float32)
    for ci, arr in enumerate((wa, wm, we)):
        for v in range(2):
            wfeat[:, 20 * ci + 10 * v: 20 * ci + 10 * v + 10] = arr[v, :, 20:30]
    s0 = wfeat[:, 9] + np.float32(eps) * wfeat[:, 19]

    # [128, NCH, B] chunk-major transposed log-rt
    lnT = np.zeros((TPAD, B), np.float32)
    lnT[:FORECAST] = lnrt.T
    lnT = np.ascontiguousarray(
        lnT.reshape(NCH, 128, B).transpose(1, 0, 2)).astype(np.float16)
    warm_mov = np.ascontiguousarray((wfeat * SCALE_OUT).T).astype(np.float16)
    s0row = (s0 * SCALE_OUT).astype(np.float16)[None, :]

    in_maps = []
    for c in range(N_CORES):
        cols = slice(c * BS, (c + 1) * BS)
        in_maps.append({
            "lnrt": np.ascontiguousarray(lnT[:, :, cols]).reshape(128, -1),
            "warm": np.ascontiguousarray(warm_mov[:, cols]),
            "s0row": np.ascontiguousarray(s0row[:, cols]),
            "cst16": cst16,
            "cst32": cst32,
        })
    return nc, in_maps


def kernel(**inputs):
    nc, in_maps = _prep(inputs)
    res = run_bass_kernel_spmd(nc, in_maps, list(range(N_CORES)))
    parts = []
    for c in range(N_CORES):
        o = np.asarray(res.results[c]["out"]).astype(np.float32)
        o = o.reshape(128, NCH, BS).transpose(1, 0, 2).reshape(TPAD, BS)
        parts.append(o[:FORECAST].T * np.float32(1.0 / SCALE_OUT))
    return np.ascontiguousarray(np.concatenate(parts, axis=0))


# revision 4
# speedup vs baseline: 1.8566x; 1.0670x over previous
"""Trainium2 Bass kernel for the CovidModel scenario forecaster (v2.1).

Math: the reference's 365-day lax.scan linearizes exactly.  With
s(tau) = a0(tau) + eps*a1(tau), s(tau) = s0 * K^tau * exp(invT * P(tau))
where P(tau) = sum_{u<=tau} ln rt_u and K = delta0 + eps*delta1.  The
three Poisson-window convolutions compose into one 28-tap linear filter
C3 on s, plus a warmup boundary term (host-folded 64x30 matrix).

Layout: TIME ON PARTITIONS (365 days -> 3 chunks of 128).  The
cumulative sum P becomes 6 blocked PE matmuls (upper-triangular L
stationary blocks for the diagonal, all-ONES for the chunk carries)
over lnrt[tau, b]; ACT computes s' = exp(invT*P + bias_tau) straight
from PSUM; the band filter is 5 Toeplitz matmuls + 1 warmup matmul; the
out pass rescales rows by e^{c*t}/256 (ACT h0 / DVE h1 split) into
bf16.  A drift renormalization s'(tau) = s(tau)/(s0*e^{c*tau}) (c =
mean daily log-growth, folded into the exp bias, the Toeplitz taps and
the row-scale) keeps all 16-bit tensors in range; every matmul moving
operand is 16-bit (1 PE cycle/row vs 4 for fp32).  The per-scenario
seed scale s0*256 rides the host-side unshard/transpose pass, together
with the ln(rt) input re-encoding and the tiny O(B*64) warmup folds.

Pipeline granularity is 1024 columns: the PSUM pool holds 4 such tiles
(8 banks), so prefix chunks, band chunks, exp and out passes all rotate
without long bank-reuse stalls.  The L weights ride the first lnrt DMA
so the first matmul issues as early as possible; the remaining
constants (+ the fp32 bias/rowscale columns, bitcast into fp16 lanes)
take one small DMA.

Sharding: batch 16384 split 8 ways, pure data parallel, no collectives.
"""

import ml_dtypes
import numpy as np

import concourse.bacc as bacc
import concourse.bass as bass
import concourse.mybir as mybir
import concourse.tile as tile
from concourse.bass_utils import run_bass_kernel_spmd

# Problem constants (fixed by the nn.Module definition)
J = 10
T_SERIAL = 5.8
B = 16384
FORECAST = 365
N_CORES = 8
BS = B // N_CORES               # 2048 scenarios per core
NCH = 3                         # 365 days -> 3 chunks of 128 (19 pad rows)
TPAD = NCH * 128
INVT = 1.0 / T_SERIAL
SCALE_OUT = 2.0 ** -8           # device output is out / (s0 * 256)
H = BS // 2                     # 1024-col pipeline half

F16 = mybir.dt.float16
BF16 = mybir.dt.bfloat16
F32 = mybir.dt.float32
Exp = mybir.ActivationFunctionType.Exp
Copy = mybir.ActivationFunctionType.Copy
BF = ml_dtypes.bfloat16

# cstB column layout: [ONES | Adiag | Abound | Aw | c32-as-fp16]
CO = slice(0, 128)
CD = slice(128, 256)
CB = slice(256, 384)
CW = slice(384, 512)
CC = slice(512, 524)
NCB = 524


def _make_host_constants(eps, delta, rho_M, rho_X, rho_G, pi_M, pi_X, pi_G,
                         lnrt_mean):
    """Fold the tiny replicated parameters into device matrices."""
    eps, delta, rho_M, rho_X, rho_G, pi_M, pi_X, pi_G = [
        np.asarray(a, np.float64)
        for a in (eps, delta, rho_M, rho_X, rho_G, pi_M, pi_X, pi_G)
    ]
    K = delta[0] + eps[0] * delta[1]
    c_drift = np.log(K) + INVT * lnrt_mean

    C3 = np.zeros(3 * (J - 1) + 1)
    for v in range(2):
        W = np.convolve(np.convolve(pi_G[v], pi_X[v]), pi_M[v])
        C3 += rho_G[v] * rho_X[v] * rho_M[v] * delta[v] * W
    C3n = C3 / K

    p = np.arange(128)[:, None]
    i = np.arange(128)[None, :]

    def band_block(off):
        A = np.zeros((128, 128))
        d = off + i - p - 3
        m = (d >= 0) & (d <= 27)
        A[m] = C3n[d[m].astype(int)] * np.exp(-c_drift * (d[m] + 3))
        return A

    # warmup boundary matrix bm [64, 30] (same folding as v1)
    bm = np.zeros((64, 30))
    for v in range(2):
        for D in range(10):
            tau = D - 9
            for t in range(1, 31):
                col = t - 1
                j = t - 1 - tau
                if 0 <= j <= 9:
                    bm[40 + 10 * v + D, col] += rho_G[v] * pi_G[v, j]
                acc = 0.0
                for jj in range(10):
                    k = t - 2 - jj - tau
                    if 0 <= k <= 9 and (t - 1 - jj) >= 1:
                        acc += pi_G[v, jj] * pi_X[v, k]
                bm[20 + 10 * v + D, col] += rho_G[v] * rho_X[v] * acc
                acc = 0.0
                for jj in range(10):
                    for k in range(10):
                        ll = t - 3 - jj - k - tau
                        if (0 <= ll <= 9 and (t - 1 - jj) >= 1
                                and (t - 2 - jj - k) >= 1):
                            acc += pi_G[v, jj] * pi_X[v, k] * pi_M[v, ll]
                bm[10 * v + D, col] += rho_G[v] * rho_X[v] * rho_M[v] * acc

    cstB = np.zeros((128, NCB), np.float16)
    cstB[:, CO] = 1.0
    cstB[:, CD] = band_block(0).astype(np.float16)
    cstB[:, CB] = band_block(128).astype(np.float16)
    Aw = np.zeros((64, 128))
    Aw[:, :30] = bm * np.exp(-c_drift * (np.arange(30) + 1))[None, :]
    cstB[:64, CW] = Aw.astype(np.float16)

    tau_idx = np.arange(1, TPAD + 1, dtype=np.float64)
    c32 = np.zeros((128, 6), np.float32)
    bias = -INVT * lnrt_mean * tau_idx               # exp bias per tau
    rs = np.exp(c_drift * tau_idx) * SCALE_OUT       # out row-scale per t row
    for c in range(NCH):
        c32[:, c] = bias[c * 128:(c + 1) * 128]
        c32[:, 3 + c] = rs[c * 128:(c + 1) * 128]
    cstB[:, CC] = c32.view(np.float16)
    return cstB, c_drift


def _build_nc():
    nc = bacc.Bacc()

    # main input: [L-weights | lnrt chunk-major], DMA'd in 1024-col pieces
    main_d = nc.dram_tensor("main", [128, 128 + NCH * BS], F16,
                            kind="ExternalInput")
    cstb_d = nc.dram_tensor("cstB", [128, NCB], F16, kind="ExternalInput")
    warm_d = nc.dram_tensor("warm", [64, BS], BF16, kind="ExternalInput")
    out_d = nc.dram_tensor("out", [128, NCH * BS], BF16, kind="ExternalOutput")

    with tile.TileContext(nc) as tc:
        with (
            tc.tile_pool(name="sb", bufs=1) as sb,
            tc.tile_pool(name="ps", bufs=4, space=bass.MemorySpace.PSUM) as ps,
        ):
            main = sb.tile([128, 128 + NCH * BS], F16, tag="main")
            # piece 0 carries the L weights + first 1024 lnrt cols
            nc.sync.dma_start(main[:, 0:128 + H], main_d[:, 0:128 + H])
            nc.sync.dma_start(main[:, 128 + H:128 + 2 * H],
                              main_d[:, 128 + H:128 + 2 * H])
            cstb = sb.tile([128, NCB], F16, tag="cstb")
            nc.sync.dma_start(cstb[:], cstb_d[:])
            for pc in range(2, 2 * NCH):     # remaining 1024-col lnrt pieces
                sl = slice(128 + pc * H, 128 + (pc + 1) * H)
                nc.sync.dma_start(main[:, sl], main_d[:, sl])
                if pc == 4:
                    warm = sb.tile([64, BS], BF16, tag="warm")
                    nc.sync.dma_start(warm[:], warm_d[:])

            Lw = main[:, 0:128]
            Ow = cstb[:, CO]
            Dw = cstb[:, CD]
            Bw = cstb[:, CB]
            Ww = cstb[:64, CW]
            c32 = cstb[:, CC].bitcast(F32)   # [128, 6] fp32 bias/rowscale

            def lnf(c, lo, hi):
                return main[:, 128 + c * BS + lo:128 + c * BS + hi]

            def mm(out_ap, w_ap, x_ap, start, stop):
                nc.tensor.matmul(out_ap, w_ap, x_ap, start=start, stop=stop)

            # --- prefix matmuls: P_c = L.lnf_c + sum_{c'<c} ONES.lnf_c'
            Ph = [[None] * 2 for _ in range(NCH)]
            for c in range(NCH):
                for h in range(2):
                    Pc = ps.tile([128, H], F32, tag="ps", name=f"P{c}h{h}")
                    Ph[c][h] = Pc
                    for j in range(2):
                        sl = slice(j * 512, (j + 1) * 512)
                        lo = h * H + j * 512
                        first = True
                        for cp in range(c):
                            mm(Pc[:, sl], Ow, lnf(cp, lo, lo + 512),
                               start=first, stop=False)
                            first = False
                        mm(Pc[:, sl], Lw, lnf(c, lo, lo + 512),
                           start=first, stop=True)

            # --- exp from PSUM (ACT), one op per 1024-col half
            s1 = []
            for c in range(NCH):
                s1c = sb.tile([128, BS], F16, tag=f"s1_{c}")
                s1.append(s1c)
                for h in range(2):
                    nc.scalar.activation(s1c[:, h * H:(h + 1) * H],
                                         Ph[c][h][:], Exp,
                                         bias=c32[:, c:c + 1],
                                         scale=float(INVT))

            # --- band matmuls + out passes (h0 on ACT, h1 on DVE)
            for c in range(NCH):
                bdh = []
                for h in range(2):
                    bd = ps.tile([128, H], F32, tag="ps", name=f"bd{c}h{h}")
                    bdh.append(bd)
                    for j in range(2):
                        sl = slice(j * 512, (j + 1) * 512)
                        lo = h * H + j * 512
                        mm(bd[:, sl], Dw, s1[c][:, lo:lo + 512],
                           start=True, stop=False)
                        if c == 0:
                            mm(bd[:, sl], Ww, warm[:, lo:lo + 512],
                               start=False, stop=True)
                        else:
                            mm(bd[:, sl], Bw, s1[c - 1][:, lo:lo + 512],
                               start=False, stop=True)

                o = sb.tile([128, BS], BF16, tag=f"o{c}")
                rs = c32[:, 3 + c:4 + c]
                nc.scalar.activation(o[:, 0:H], bdh[0][:], Copy,
                                     bias=0.0, scale=rs)
                nc.vector.tensor_scalar_mul(o[:, H:BS], bdh[1][:], rs)
                nc.sync.dma_start(out_d[:, c * BS + H:(c + 1) * BS],
                                  o[:, H:BS])
                nc.sync.dma_start(out_d[:, c * BS:c * BS + H], o[:, 0:H])

    nc.compile()
    return nc


_CACHE = {}


def _prep(inputs):
    r_t = np.asarray(inputs["r_t"], np.float32)
    wa = np.asarray(inputs["warmup_asymp"], np.float32)
    wm = np.asarray(inputs["warmup_mild"], np.float32)
    we = np.asarray(inputs["warmup_extreme"], np.float32)
    eps = float(np.asarray(inputs["eps"], np.float64)[0])

    lnrt = np.log(r_t)                       # [B, 365] fp32
    lnrt_mean = float(lnrt.astype(np.float64).mean())

    cstB, _ = _make_host_constants(
        inputs["eps"], inputs["delta"], inputs["rho_M"], inputs["rho_X"],
        inputs["rho_G"], inputs["pi_M"], inputs["pi_X"], inputs["pi_G"],
        lnrt_mean)

    if "nc" not in _CACHE:
        _CACHE["nc"] = _build_nc()
    nc = _CACHE["nc"]

    # warmup features: last 10 days of each compartment, (B, 64)
    wfeat = np.zeros((B, 64), np.float32)
    for ci, arr in enumerate((wa, wm, we)):
        for v in range(2):
            wfeat[:, 20 * ci + 10 * v: 20 * ci + 10 * v + 10] = arr[v, :, 20:30]
    s0 = wfeat[:, 9] + np.float32(eps) * wfeat[:, 19]

    # [128, 128 + NCH*B]: L weights + chunk-major transposed log-rt
    lnT = np.zeros((TPAD, B), np.float32)
    lnT[:FORECAST] = lnrt.T
    lnT = lnT.reshape(NCH, 128, B).transpose(1, 0, 2)   # [128, NCH, B]
    mainL = np.triu(np.ones((128, 128), np.float16))
    warm_mov = (wfeat / s0[:, None]).T                  # [64, B]

    in_maps = []
    for c in range(N_CORES):
        cols = slice(c * BS, (c + 1) * BS)
        main = np.empty((128, 128 + NCH * BS), np.float16)
        main[:, :128] = mainL
        main[:, 128:] = lnT[:, :, cols].reshape(128, -1)
        in_maps.append({
            "main": main,
            "cstB": cstB,
            "warm": np.ascontiguousarray(warm_mov[:, cols]).astype(BF),
        })
    return nc, in_maps, s0


def kernel(**inputs):
    nc, in_maps, s0 = _prep(inputs)
    res = run_bass_kernel_spmd(nc, in_maps, list(range(N_CORES)))
    parts = []
    for c in range(N_CORES):
        o = np.asarray(res.results[c]["out"]).astype(np.float32)
        o = o.reshape(128, NCH, BS).transpose(1, 0, 2).reshape(TPAD, BS)
        seed = s0[c * BS:(c + 1) * BS, None] * np.float32(1.0 / SCALE_OUT)
        parts.append(o[:FORECAST].T * seed)
    return np.ascontiguousarray(np.concatenate(parts, axis=0))


# revision 6
# speedup vs baseline: 2.1671x; 1.1673x over previous
"""Trainium2 Bass kernel for the CovidModel scenario forecaster (v2.1).

Math: the reference's 365-day lax.scan linearizes exactly.  With
s(tau) = a0(tau) + eps*a1(tau), s(tau) = s0 * K^tau * exp(invT * P(tau))
where P(tau) = sum_{u<=tau} ln rt_u and K = delta0 + eps*delta1.  The
three Poisson-window convolutions compose into one 28-tap linear filter
C3 on s, plus a warmup boundary term (host-folded 64x30 matrix).

Layout: TIME ON PARTITIONS (365 days -> 3 chunks of 128).  The
cumulative sum P becomes 6 blocked PE matmuls (upper-triangular L
stationary blocks for the diagonal, all-ONES for the chunk carries)
over lnrt[tau, b]; ACT computes s' = exp(invT*P + bias_tau) straight
from PSUM; the band filter is 5 Toeplitz matmuls + 1 warmup matmul; the
out pass rescales rows by e^{c*t}/256 (ACT h0 / DVE h1 split) into
bf16.  A drift renormalization s'(tau) = s(tau)/(s0*e^{c*tau}) (c =
mean daily log-growth, folded into the exp bias, the Toeplitz taps and
the row-scale) keeps all 16-bit tensors in range; every matmul moving
operand is 16-bit (1 PE cycle/row vs 4 for fp32).  The per-scenario
seed scale s0*256 rides the host-side unshard/transpose pass, together
with the ln(rt) input re-encoding and the tiny O(B*64) warmup folds.

Pipeline granularity is 1024 columns: the PSUM pool holds 4 such tiles
(8 banks), so prefix chunks, band chunks, exp and out passes all rotate
without long bank-reuse stalls.  The L weights ride the first lnrt DMA
so the first matmul issues as early as possible; the remaining
constants (+ the fp32 bias/rowscale columns, bitcast into fp16 lanes)
take one small DMA.

Sharding: batch 16384 split 8 ways, pure data parallel, no collectives.
"""

import ml_dtypes
import numpy as np

import concourse.bacc as bacc
import concourse.bass as bass
import concourse.mybir as mybir
import concourse.tile as tile
from concourse.bass_utils import run_bass_kernel_spmd

# Problem constants (fixed by the nn.Module definition)
J = 10
T_SERIAL = 5.8
B = 16384
FORECAST = 365
N_CORES = 8
BS = B // N_CORES               # 2048 scenarios per core
NCH = 3                         # 365 days -> 3 chunks of 128 (19 pad rows)
TPAD = NCH * 128
INVT = 1.0 / T_SERIAL
SCALE_OUT = 2.0 ** -8           # device output is out / (s0 * 256)
H = BS // 2                     # 1024-col pipeline half

F16 = mybir.dt.float16
BF16 = mybir.dt.bfloat16
F32 = mybir.dt.float32
Exp = mybir.ActivationFunctionType.Exp
Copy = mybir.ActivationFunctionType.Copy
BF = ml_dtypes.bfloat16

# cstB column layout: [ONES | Adiag | Abound | Aw | c32-as-fp16]
CO = slice(0, 128)
CD = slice(128, 256)
CB = slice(256, 384)
CW = slice(384, 512)
CC = slice(512, 524)
NCB = 524


def _make_host_constants(eps, delta, rho_M, rho_X, rho_G, pi_M, pi_X, pi_G,
                         lnrt_mean):
    """Fold the tiny replicated parameters into device matrices."""
    eps, delta, rho_M, rho_X, rho_G, pi_M, pi_X, pi_G = [
        np.asarray(a, np.float64)
        for a in (eps, delta, rho_M, rho_X, rho_G, pi_M, pi_X, pi_G)
    ]
    K = delta[0] + eps[0] * delta[1]
    c_drift = np.log(K) + INVT * lnrt_mean

    C3 = np.zeros(3 * (J - 1) + 1)
    for v in range(2):
        W = np.convolve(np.convolve(pi_G[v], pi_X[v]), pi_M[v])
        C3 += rho_G[v] * rho_X[v] * rho_M[v] * delta[v] * W
    C3n = C3 / K

    p = np.arange(128)[:, None]
    i = np.arange(128)[None, :]

    def band_block(off):
        A = np.zeros((128, 128))
        d = off + i - p - 3
        m = (d >= 0) & (d <= 27)
        A[m] = C3n[d[m].astype(int)] * np.exp(-c_drift * (d[m] + 3))
        return A

    # warmup boundary matrix bm [64, 30] (same folding as v1)
    bm = np.zeros((64, 30))
    for v in range(2):
        for D in range(10):
            tau = D - 9
            for t in range(1, 31):
                col = t - 1
                j = t - 1 - tau
                if 0 <= j <= 9:
                    bm[40 + 10 * v + D, col] += rho_G[v] * pi_G[v, j]
                acc = 0.0
                for jj in range(10):
                    k = t - 2 - jj - tau
                    if 0 <= k <= 9 and (t - 1 - jj) >= 1:
                        acc += pi_G[v, jj] * pi_X[v, k]
                bm[20 + 10 * v + D, col] += rho_G[v] * rho_X[v] * acc
                acc = 0.0
                for jj in range(10):
                    for k in range(10):
                        ll = t - 3 - jj - k - tau
                        if (0 <= ll <= 9 and (t - 1 - jj) >= 1
                                and (t - 2 - jj - k) >= 1):
                            acc += pi_G[v, jj] * pi_X[v, k] * pi_M[v, ll]
                bm[10 * v + D, col] += rho_G[v] * rho_X[v] * rho_M[v] * acc

    cstB = np.zeros((128, NCB), np.float16)
    cstB[:, CO] = 1.0
    cstB[:, CD] = band_block(0).astype(np.float16)
    cstB[:, CB] = band_block(128).astype(np.float16)
    Aw = np.zeros((64, 128))
    Aw[:, :30] = bm * np.exp(-c_drift * (np.arange(30) + 1))[None, :]
    cstB[:64, CW] = Aw.astype(np.float16)

    tau_idx = np.arange(1, TPAD + 1, dtype=np.float64)
    c32 = np.zeros((128, 6), np.float32)
    bias = -INVT * lnrt_mean * tau_idx               # exp bias per tau
    rs = np.exp(c_drift * tau_idx) * SCALE_OUT       # out row-scale per t row
    for c in range(NCH):
        c32[:, c] = bias[c * 128:(c + 1) * 128]
        c32[:, 3 + c] = rs[c * 128:(c + 1) * 128]
    cstB[:, CC] = c32.view(np.float16)
    return cstB, c_drift


def _build_nc():
    nc = bacc.Bacc()

    # main input: [L-weights | lnrt chunk-major], DMA'd in 1024-col pieces
    main_d = nc.dram_tensor("main", [128, 128 + NCH * BS], F16,
                            kind="ExternalInput")
    cstb_d = nc.dram_tensor("cstB", [128, NCB], F16, kind="ExternalInput")
    warm_d = nc.dram_tensor("warm", [64, BS], BF16, kind="ExternalInput")
    out_d = nc.dram_tensor("out", [128, NCH * BS], BF16, kind="ExternalOutput")

    N_DUMMY = 7                  # PE p-state warmup matmuls during input DMA

    with tile.TileContext(nc) as tc:
        with (
            tc.tile_pool(name="sb", bufs=1) as sb,
            tc.tile_pool(name="ps", bufs=4, space=bass.MemorySpace.PSUM) as ps,
        ):
            main = sb.tile([128, 128 + NCH * BS], F16, tag="main")
            # piece 0 carries the L weights + first 1024 lnrt cols
            nc.sync.dma_start(main[:, 0:128 + H], main_d[:, 0:128 + H])
            nc.sync.dma_start(main[:, 128 + H:128 + 2 * H],
                              main_d[:, 128 + H:128 + 2 * H])
            cstb = sb.tile([128, NCB], F16, tag="cstb")
            nc.sync.dma_start(cstb[:], cstb_d[:])
            for pc in range(2, 2 * NCH):     # remaining 1024-col lnrt pieces
                sl = slice(128 + pc * H, 128 + (pc + 1) * H)
                nc.sync.dma_start(main[:, sl], main_d[:, sl])
            warm = sb.tile([64, BS], BF16, tag="warm")
            nc.sync.dma_start(warm[:], warm_d[:])

            # --- PE p-state warmup + early ACT table load on scratch data
            scr = sb.tile([128, 512], F16, tag="scr")
            nc.gpsimd.memset(scr[:], 1.0)
            scr1 = sb.tile([128, 1], F32, tag="scr1")
            nc.scalar.activation(scr1[:], scr[:, 0:1], Exp, bias=0.0,
                                 scale=1.0)
            dps = ps.tile([128, H], F32, tag="ps", name="dummy")
            for k in range(N_DUMMY):
                nc.tensor.matmul(dps[:, 0:512], scr[:, 0:128], scr[:],
                                 start=True, stop=True)

            Lw = main[:, 0:128]
            Ow = cstb[:, CO]
            Dw = cstb[:, CD]
            Bw = cstb[:, CB]
            Ww = cstb[:64, CW]
            c32 = cstb[:, CC].bitcast(F32)   # [128, 6] fp32 bias/rowscale

            def lnf(c, lo, hi):
                return main[:, 128 + c * BS + lo:128 + c * BS + hi]

            def mm(out_ap, w_ap, x_ap, start, stop):
                nc.tensor.matmul(out_ap, w_ap, x_ap, start=start, stop=stop,
                                 skip_group_check=True)

            # --- prefix matmuls: P_c = L.lnf_c + sum_{c'<c} ONES.lnf_c'
            # psum tiles; allocation order fixes the 4-slot rotation
            Ph = [[ps.tile([128, H], F32, tag="ps", name=f"P{c}h{h}")
                   for h in range(2)] for c in range(NCH)]

            def quarters(c):
                for h in range(2):
                    for j in range(2):
                        yield h, slice(j * 512, (j + 1) * 512), h * H + j * 512

            # arrival-ordered emission: lnf0-only work first, L parts when
            # their chunk has certainly landed
            for c, h, sl, lo in (
                [(0, h, sl, lo) for h, sl, lo in quarters(0)]):
                mm(Ph[0][h][:, sl], Lw, lnf(0, lo, lo + 512), True, True)
            for h, sl, lo in quarters(1):    # P1 += ONES.lnf0
                mm(Ph[1][h][:, sl], Ow, lnf(0, lo, lo + 512), True, False)
            for h, sl, lo in quarters(2):    # P2 += ONES.lnf0
                mm(Ph[2][h][:, sl], Ow, lnf(0, lo, lo + 512), True, False)
            for h, sl, lo in quarters(1):    # P1 += L.lnf1
                mm(Ph[1][h][:, sl], Lw, lnf(1, lo, lo + 512), False, True)
            for h, sl, lo in quarters(2):    # P2 += ONES.lnf1
                mm(Ph[2][h][:, sl], Ow, lnf(1, lo, lo + 512), False, False)
            for h, sl, lo in quarters(2):    # P2 += L.lnf2
                mm(Ph[2][h][:, sl], Lw, lnf(2, lo, lo + 512), False, True)

            # --- exp from PSUM (ACT), one op per 1024-col half
            s1 = []
            for c in range(NCH):
                s1c = sb.tile([128, BS], F16, tag=f"s1_{c}")
                s1.append(s1c)
                for h in range(2):
                    nc.scalar.activation(s1c[:, h * H:(h + 1) * H],
                                         Ph[c][h][:], Exp,
                                         bias=c32[:, c:c + 1],
                                         scale=float(INVT))

            # --- band matmuls (diag taps first, boundary/warmup second)
            bdt = []
            for c in range(NCH):
                bdh = [ps.tile([128, H], F32, tag="ps", name=f"bd{c}h{h}")
                       for h in range(2)]
                bdt.append(bdh)
                for h, sl, lo in quarters(c):
                    mm(bdh[h][:, sl], Dw, s1[c][:, lo:lo + 512], True, False)
                for h, sl, lo in quarters(c):
                    if c == 0:
                        mm(bdh[h][:, sl], Ww, warm[:, lo:lo + 512],
                           False, True)
                    else:
                        mm(bdh[h][:, sl], Bw, s1[c - 1][:, lo:lo + 512],
                           False, True)

            # --- out passes: rescale psum -> bf16 (ACT / DVE / Pool split)
            outs = []
            for c in range(NCH):
                o = sb.tile([128, BS], BF16, tag=f"o{c}")
                outs.append(o)
            for c in range(NCH):
                rs = c32[:, 3 + c:4 + c]
                nc.scalar.activation(outs[c][:, 0:H], bdt[c][0][:], Copy,
                                     bias=0.0, scale=rs)
                nc.vector.tensor_scalar_mul(outs[c][:, H:BS], bdt[c][1][:],
                                            rs)
            for c in range(NCH):
                nc.sync.dma_start(out_d[:, c * BS + H:(c + 1) * BS],
                                  outs[c][:, H:BS])
                nc.sync.dma_start(out_d[:, c * BS:c * BS + H],
                                  outs[c][:, 0:H])

    nc.compile()
    return nc


_CACHE = {}


def _prep(inputs):
    r_t = np.asarray(inputs["r_t"], np.float32)
    wa = np.asarray(inputs["warmup_asymp"], np.float32)
    wm = np.asarray(inputs["warmup_mild"], np.float32)
    we = np.asarray(inputs["warmup_extreme"], np.float32)
    eps = float(np.asarray(inputs["eps"], np.float64)[0])

    lnrt = np.log(r_t)                       # [B, 365] fp32
    lnrt_mean = float(lnrt.astype(np.float64).mean())

    cstB, _ = _make_host_constants(
        inputs["eps"], inputs["delta"], inputs["rho_M"], inputs["rho_X"],
        inputs["rho_G"], inputs["pi_M"], inputs["pi_X"], inputs["pi_G"],
        lnrt_mean)

    if "nc" not in _CACHE:
        _CACHE["nc"] = _build_nc()
    nc = _CACHE["nc"]

    # warmup features: last 10 days of each compartment, (B, 64)
    wfeat = np.zeros((B, 64), np.float32)
    for ci, arr in enumerate((wa, wm, we)):
        for v in range(2):
            wfeat[:, 20 * ci + 10 * v: 20 * ci + 10 * v + 10] = arr[v, :, 20:30]
    s0 = wfeat[:, 9] + np.float32(eps) * wfeat[:, 19]

    # [128, 128 + NCH*B]: L weights + chunk-major transposed log-rt
    lnT = np.zeros((TPAD, B), np.float32)
    lnT[:FORECAST] = lnrt.T
    lnT = lnT.reshape(NCH, 128, B).transpose(1, 0, 2)   # [128, NCH, B]
    mainL = np.triu(np.ones((128, 128), np.float16))
    warm_mov = (wfeat / s0[:, None]).T                  # [64, B]

    in_maps = []
    for c in range(N_CORES):
        cols = slice(c * BS, (c + 1) * BS)
        main = np.empty((128, 128 + NCH * BS), np.float16)
        main[:, :128] = mainL
        main[:, 128:] = lnT[:, :, cols].reshape(128, -1)
        in_maps.append({
            "main": main,
            "cstB": cstB,
            "warm": np.ascontiguousarray(warm_mov[:, cols]).astype(BF),
        })
    return nc, in_maps, s0


def kernel(**inputs):
    nc, in_maps, s0 = _prep(inputs)
    res = run_bass_kernel_spmd(nc, in_maps, list(range(N_CORES)))
    parts = []
    for c in range(N_CORES):
        o = np.asarray(res.results[c]["out"]).astype(np.float32)
        o = o.reshape(128, NCH, BS).transpose(1, 0, 2).reshape(TPAD, BS)
        seed = s0[c * BS:(c + 1) * BS, None] * np.float32(1.0 / SCALE_OUT)
        parts.append(o[:FORECAST].T * seed)
    return np.ascontiguousarray(np.concatenate(parts, axis=0))
